# revision 35
# baseline (speedup 1.0000x reference)
"""Trainium2 Bass kernel for nn_CausalFieldAttention (v3).

Shapes (hardcoded): B=4, N=4096, D=1024, H=16, hd=64, G=512, sigma=3.

Reference computation (q-projection is computed but unused -> skipped):
    k  = x @ k_w.T + k_b                      (B,N,D) -> heads (B,H,N,hd)
    v  = x @ v_w.T + v_b
    wv = v * ||k||_head
    field = segment_sum(wv, field_idx, G)     scatter tokens -> G bins
    conv  = circular_conv(field, causal_ker)  (exact circulant)
    y  = conv[field_idx]                      gather bins -> tokens
    out = y @ out_w.T + out_b
Device computes A = conv @ ow at bin granularity; host replicates bin
rows to tokens (pure gather) and sums the two head-group partials.

Device strategy: 8 cores = 4 batches x 2 head-groups (512 channels each).
v3 changes vs v2 (145.7us):
  - Scatter at pair (128-bin) granularity: one [128tok->128bin, 512] MM
    per (tile, gt) instead of per 64-bin half => ~25% fewer scatter MMs,
    full 128-wide PE columns.  The last bin-tile (gt3) keeps fine
    granularity in three column segments (bins 384-447 / 448-479 /
    480-511) so the structural tail is only the final 32-bin segment.
  - Conv contracts K=128 over full pairs (K=64/K=32 for gt3 segments)
    and uses four uniform W=128 g-ranges => 44 cheap N=128 conv MMs and
    32 A-proj MMs (vs 72 + 40).
  - Tail range [128,256) accumulates its conv directly in PSUM across
    t28/t30/t31 partial triggers: no SBUF pre-accumulate + DVE add, and
    the post-last-token work is 4 K=32 conv MMs + 8 A MMs + 0.5MB DMA.
  - DMA descriptor payloads: x8/xb loaded as 2-tile pairs (2/4KB rows),
    kw/vw/ct/ow as whole tensors (4-8KB rows), S-blocks host-packed per
    gt (2.25KB rows).  Startup DMAs are deadline-ordered across the
    three queues; dependency-free warm-up MMs on the first-landed x8
    pair pull the PE out of its cold HAM state during the DMA ramp.
"""

import os
import sys
from contextlib import ExitStack

import numpy as np

for _p in ("/opt/trn_rl_repo", "/root/.axon_site/_ro/trn_rl_repo"):
    if os.path.isdir(_p) and _p not in sys.path:
        sys.path.append(_p)

import concourse.bacc as bacc
import concourse.mybir as mybir
import concourse.tile as tile
from concourse.bass_utils import run_bass_kernel_spmd

B, N, D = 4, 4096, 1024
H, HD, G = 16, 64, 512
SIGMA = 3.0
P = 128
KT = D // P          # 8 contraction tiles over D
TT = N // P          # 32 token tiles
GT = G // P          # 4 bin pair-tiles
CLOC = 512           # channels per core (8 heads)
HLOC = CLOC // HD    # 8 heads per core
ECH = D // 512       # 2 chunks of out-channels for 512-wide psum
NCORES = 8
LEAD = 4             # k-projection runs this many tiles ahead of v

F32 = mybir.dt.float32
BF16 = mybir.dt.bfloat16
FP8 = mybir.dt.float8e4
NP_BF16 = mybir.dt.np(BF16)
NP_FP8 = mybir.dt.np(FP8)
KSCALE = 32.0   # k-weights are scaled x32 into fp8's normal range; the
                # resulting 32x on ||k|| is compensated exactly (power of
                # two) by scaling the conv matrix by 1/32.

# gt3 column segments (pair-partition coords; bin = 384 + p)
SEGS = ((0, 96), (96, 128))

# set by test harness to capture a profile; kernel() stores results here
TRACE = False
LAST_RESULT = None


def _field_idx():
    # exactly mirrors the reference (fp32 div then mul, trunc, clip)
    pos = np.arange(N, dtype=np.float32) / np.float32(N - 1) * np.float32(G - 1)
    return np.clip(pos.astype(np.int32), 0, G - 1)


def _causal_kernel():
    i = np.arange(G)
    dist = np.abs(i - G // 2)
    ker = np.where(i >= G // 2, 0.0, np.exp(-dist / SIGMA)).astype(np.float32)
    ker = ker / (ker.sum() + 1e-8)
    return ker


def _plans():
    idx = _field_idx()
    ker = _causal_kernel()
    gg = (np.arange(G)[None, :] - np.arange(G)[:, None]) % G
    CTm = ker[gg].astype(np.float32)      # CTm[f, g] = ker[(g-f)%G]

    Smat = np.zeros((N, G), np.float32)
    Smat[np.arange(N), idx] = 1.0

    counts = np.bincount(idx, minlength=G)           # tokens per bin
    tok_start = np.concatenate([[0], np.cumsum(counts)])

    # kernel support: ker[m] > 1e-12 for m in [mlo, mhi] = [176, 255]
    nz = np.where(ker > 1e-12)[0]
    mlo, mhi = int(nz.min()), int(nz.max())

    # scatter segments: (gt, plo, phi) with bin range [gt*128+plo, gt*128+phi)
    segments = [(gt, 0, P) for gt in range(3)]
    segments += [(3, lo, hi) for lo, hi in SEGS]

    def tiles_touching(b0, b1):
        ts = []
        for t in range(TT):
            bt = idx[t * P:(t + 1) * P]
            if np.any((bt >= b0) & (bt < b1)):
                ts.append(t)
        return ts

    seg_tiles = {}
    for gt, plo, phi in segments:
        seg_tiles[(gt, plo, phi)] = tiles_touching(gt * P + plo, gt * P + phi)

    tile_gts = []
    for t in range(TT):
        bt = idx[t * P:(t + 1) * P]
        tile_gts.append(sorted(set((bt // P).tolist())))

    # ranges: conv[g] needs field bins [(g-mhi)%G, (g-mlo)%G].
    def contributors(glo, ghi):
        need = set()
        for g in range(glo, ghi):
            for m in range(mlo, mhi + 1):
                need.add((g - m) % G)
        out = []
        for gt, plo, phi in segments:
            if any((gt * P + p) in need for p in range(plo, phi)):
                out.append((gt, plo, phi))
        return out

    ranges = []
    for ri, glo in enumerate((256, 384, 0, 128)):
        ghi = glo + P
        cons = contributors(glo, ghi)
        trig = max(max(seg_tiles[c]) for c in cons)
        chunks = []
        b = glo
        while b < ghi:
            c = int(counts[b])
            nb = 1
            while b + nb < ghi and int(counts[b + nb]) == c:
                nb += 1
            chunks.append((int(tok_start[b]), b, nb, c))
            b += nb
        ranges.append({"ri": ri, "glo": glo, "ghi": ghi, "cons": cons,
                       "trigger": trig, "chunks": chunks})

    # S-block list per gt (host packs these in order)
    sblocks = {gt: [t for t in range(TT) if gt in tile_gts[t]]
               for gt in range(GT)}

    return {
        "idx": idx, "CTm": CTm, "Smat": Smat,
        "segments": segments, "seg_tiles": seg_tiles,
        "tile_gts": tile_gts, "ranges": ranges, "sblocks": sblocks,
        "counts": counts,
    }


def _build_program(with_kb, with_vb, pl):
    tile_gts = pl["tile_gts"]
    seg_tiles = pl["seg_tiles"]
    segments = pl["segments"]
    ranges = pl["ranges"]
    sblocks = pl["sblocks"]
    NR = len(ranges)
    TPAIRS = TT // 2

    nc = bacc.Bacc("TRN2", target_bir_lowering=False, debug=False,
                   num_devices=NCORES)
    # host-permuted layouts; x8/xb pack TWO token tiles per 128-partition
    # block so every DMA row is 2/4KB.
    xTt = nc.dram_tensor("xTt", [TPAIRS * P, 2 * KT * P], BF16,
                         kind="ExternalInput").ap()
    x8t = nc.dram_tensor("x8t", [TPAIRS * P, 2 * KT * P], FP8,
                         kind="ExternalInput").ap()
    kwt = nc.dram_tensor("kwt", [P, KT * CLOC], FP8, kind="ExternalInput").ap()
    vwt = nc.dram_tensor("vwt", [P, KT * CLOC], BF16, kind="ExternalInput").ap()
    owt = nc.dram_tensor("owt", [P, GT * D], BF16, kind="ExternalInput").ap()
    ctt = nc.dram_tensor("ctt", [P, GT * G], BF16, kind="ExternalInput").ap()
    smp = {gt: nc.dram_tensor(f"smp{gt}", [P, len(sblocks[gt]) * P], FP8,
                              kind="ExternalInput").ap() for gt in range(GT)}
    kb = nc.dram_tensor("kb", [1, CLOC], BF16, kind="ExternalInput").ap() if with_kb else None
    vb = nc.dram_tensor("vb", [1, CLOC], BF16, kind="ExternalInput").ap() if with_vb else None
    ones_d = (nc.dram_tensor("ones", [1, P], BF16, kind="ExternalInput").ap()
              if (with_kb or with_vb) else None)
    aout = nc.dram_tensor("aout", [NR * P, D], BF16,
                          kind="ExternalOutput").ap()

    with tile.TileContext(nc) as tc, ExitStack() as es:
        cpool = es.enter_context(tc.tile_pool(name="const", bufs=1))

        kw_sb = cpool.tile([P, KT, CLOC], FP8)
        vw_sb = cpool.tile([P, KT, CLOC], BF16)
        ow_sb = cpool.tile([P, GT, D], BF16)
        ct_sb = cpool.tile([P, GT, G], BF16)        # [f%128, f//128, g]
        field_sb = cpool.tile([P, GT, CLOC], BF16)  # [f%128, f//128, ch]
        convT_sb = cpool.tile([P, GT, G], BF16)     # [ch%128, ch//128, g]
        A_sb = cpool.tile([P, NR, D], BF16)         # [bin-glo(r), r, e]
        sm_sb = {gt: cpool.tile([P, len(sblocks[gt]), P], FP8,
                                name=f"sm{gt}") for gt in range(GT)}
        if with_kb or with_vb:
            ones_sb = cpool.tile([1, P], BF16)
            nc.sync.dma_start(ones_sb[:], ones_d[:])
        if with_kb:
            kb_sb = cpool.tile([1, CLOC], BF16)
            nc.sync.dma_start(kb_sb[:], kb[:])
        if with_vb:
            vb_sb = cpool.tile([1, CLOC], BF16)
            nc.sync.dma_start(vb_sb[:], vb[:])

        xpool = es.enter_context(tc.tile_pool(name="xin", bufs=3))
        x8pool = es.enter_context(tc.tile_pool(name="x8in", bufs=4))
        smpool = es.enter_context(tc.tile_pool(name="small", bufs=3))
        wvpool = es.enter_context(tc.tile_pool(name="wv", bufs=3))
        # 4-deep k/v ring: v(t+1) reuses the slot freed by square(k(t+3)) on
        # ACT; at depth 3 a momentarily busy ACT stalls the PE v-stream.
        ps_kv = es.enter_context(tc.tile_pool(name="ps_kv", bufs=4, space="PSUM"))
        ps_f = es.enter_context(tc.tile_pool(name="ps_f", bufs=2, space="PSUM"))
        ps_m = es.enter_context(tc.tile_pool(name="ps_m", bufs=2, space="PSUM"))

        eng_flip = [0]

        def flip_copy(dst, src):
            # alternate DVE/ACT for PSUM->SBUF traffic
            if eng_flip[0] % 2 == 0:
                nc.vector.tensor_copy(dst, src)
            else:
                nc.scalar.copy(dst, src)
            eng_flip[0] += 1

        # ---- startup DMA plan: deadline-ordered across three queues ----
        x8_pairs = {}
        xb_pairs = {}

        def x8p(tp):
            t8 = x8pool.tile([P, 2, KT, P], FP8, tag="x8blk", bufs=4,
                             name=f"x8p{tp % 4}")
            x8_pairs[tp] = t8
            return (t8[:], x8t[tp * P:(tp + 1) * P, :]
                    .rearrange("p (j kt c) -> p j kt c", j=2, kt=KT))

        def xbp(tp):
            tb = xpool.tile([P, 2, KT, P], BF16, tag="xblk", bufs=3,
                            name=f"xbp{tp % 3}")
            xb_pairs[tp] = tb
            return (tb[:], xTt[tp * P:(tp + 1) * P, :]
                    .rearrange("p (j kt c) -> p j kt c", j=2, kt=KT))

        # first x8/xb pairs land as two per-tile chunks so the earliest
        # matmuls gate on the minimum bytes; kw/vw land in halves.
        x8p(0)
        xbp(0)
        p80 = x8_pairs[0]
        pb0 = xb_pairs[0]
        x8r0 = x8t[0:P, :].rearrange("p (j kt c) -> p j kt c", j=2, kt=KT)
        xbr0 = xTt[0:P, :].rearrange("p (j kt c) -> p j kt c", j=2, kt=KT)
        kwr = kwt.rearrange("p (kt c) -> p kt c", kt=KT)
        vwr = vwt.rearrange("p (kt c) -> p kt c", kt=KT)
        plan = {
            nc.sync: [(p80[:, 0], x8r0[:, 0]),
                      (p80[:, 1], x8r0[:, 1]),
                      x8p(1),
                      (vw_sb[:, 0:4], vwr[:, 0:4])],
            nc.scalar: [(kw_sb[:, 0:4], kwr[:, 0:4]),
                        (kw_sb[:, 4:8], kwr[:, 4:8]),
                        x8p(2),
                        (sm_sb[0][:], smp[0].rearrange("p (i c) -> p i c", c=P))],
            nc.gpsimd: [(pb0[:, 0], xbr0[:, 0]),
                        (pb0[:, 1], xbr0[:, 1]),
                        (vw_sb[:, 4:8], vwr[:, 4:8]),
                        (sm_sb[1][:], smp[1].rearrange("p (i c) -> p i c", c=P))],
        }
        for eng, items in plan.items():
            for dst, srcap in items:
                eng.dma_start(dst, srcap)

        # warm-up: dependency-free matmuls on the first x8 chunk bridge the
        # DMA ramp and un-throttle the PE's HAM clock before real work.
        warm_ps = ps_m.tile([P, CLOC], F32, tag="mid", name="warm")
        for i in range(6):
            nc.tensor.matmul(warm_ps[:], x8_pairs[0][:, 0, 0, :],
                             x8_pairs[0][:, 0, 0:4, :]
                             .rearrange("p kt c -> p (kt c)"),
                             start=True, stop=True)

        field_ps = {}
        seg_open = {}

        def emit_scatter(t, wv):
            for gt in tile_gts[t]:
                if gt not in field_ps:
                    field_ps[gt] = ps_f.tile([P, CLOC], F32, tag="fld",
                                             name=f"fld{gt}")
                si = sblocks[gt].index(t)
                st = sm_sb[gt][:, si, :]
                if gt < 3:
                    tts = seg_tiles[(gt, 0, P)]
                    nc.tensor.matmul(field_ps[gt][:], st, wv[:],
                                     start=(t == tts[0]), stop=(t == tts[-1]))
                    if t == tts[-1]:
                        flip_copy(field_sb[:, gt, :], field_ps[gt][:])
                        del field_ps[gt]
                else:
                    for plo, phi in SEGS:
                        tts = seg_tiles[(3, plo, phi)]
                        if t not in tts:
                            continue
                        kw = {}
                        if plo > 0:
                            kw["tile_position"] = (0, plo)
                        nc.tensor.matmul(field_ps[3][plo:phi, :],
                                         st[:, plo:phi], wv[:],
                                         start=(t == tts[0]),
                                         stop=(t == tts[-1]), **kw)
                        if t == tts[-1]:
                            flip_copy(field_sb[plo:phi, 3, :],
                                      field_ps[3][plo:phi, :])
                            seg_open[(plo, phi)] = True
                            if len(seg_open) == len(SEGS):
                                del field_ps[3]

        def conv_mms(cv, r, cons, first, last, single_start=False):
            # cv[ch, ct*W + (g-glo)] accumulated over contributor segments.
            # start=True clears has_written for the WHOLE bank, so a
            # staggered (multi-burst) accumulation must issue it exactly
            # once: later first-writes overwrite naturally via
            # has_written=0, later repeat-writes accumulate.
            glo, ghi = r["glo"], r["ghi"]
            W = ghi - glo
            for ct in range(GT):
                for j, (gt, plo, phi) in enumerate(cons):
                    kw = {}
                    if plo in (64, 96):
                        kw["tile_position"] = (plo, 0)
                    if single_start:
                        st = first and ct == 0 and j == 0
                    else:
                        st = first and j == 0
                    nc.tensor.matmul(
                        cv[:, ct * W:(ct + 1) * W],
                        field_sb[plo:phi, gt, ct * P:(ct + 1) * P],
                        ct_sb[plo:phi, gt, glo:ghi],
                        start=st,
                        stop=(last and j == len(cons) - 1), **kw)

        def job_A(r):
            ri, glo, ghi = r["ri"], r["glo"], r["ghi"]
            W = ghi - glo
            for ec in range(ECH):
                esl = slice(ec * 512, (ec + 1) * 512)
                pa = ps_m.tile([P, 512], F32, tag="mid")
                for ct in range(GT):
                    nc.tensor.matmul(pa[0:W, :],
                                     convT_sb[:, ct, glo:ghi],
                                     ow_sb[:, ct, esl],
                                     start=(ct == 0), stop=(ct == GT - 1))
                # the tail range drains in half-chunks so the last DMA
                # starts as early as possible
                nch = 2 if ri == 3 else 1
                for c in range(nch):
                    cw = 512 // nch
                    el = slice(ec * 512 + c * cw, ec * 512 + (c + 1) * cw)
                    flip_copy(A_sb[0:W, ri, el], pa[0:W, c * cw:(c + 1) * cw])
                    eng = nc.sync if (ec + c) % 2 == 0 else nc.scalar
                    eng.dma_start(aout[ri * P:ri * P + W, el],
                                  A_sb[0:W, ri, el])

        def fin_convT(r, cv):
            glo, ghi = r["glo"], r["ghi"]
            W = ghi - glo
            for lo, hi in ((0, 2), (2, 4)):
                flip_copy(convT_sb[:, lo:hi, glo:ghi],
                          cv[:, lo * W:hi * W].rearrange("p (ct w) -> p ct w",
                                                         w=W))

        def warm_tail(n):
            # dependency-free matmuls on long-resident operands keep the
            # PE's HAM clock at full rate across tail copy-latency bubbles
            wt = ps_kv.tile([P, CLOC], F32, tag="kv", name="warm")
            for i in range(n):
                nc.tensor.matmul(wt[:], field_sb[:, 0, 0:P],
                                 ct_sb[:, 0, 0:CLOC],
                                 start=(i == 0), stop=(i == n - 1))

        def job_range(r, warm=0):
            cv = ps_m.tile([P, 512], F32, tag="mid")
            conv_mms(cv, r, r["cons"], True, True)
            fin_convT(r, cv)
            if warm:
                warm_tail(warm)
            job_A(r)

        # tail range R3 [128,256): conv accumulates in a parked PSUM bank
        # across its staggered contributor triggers.
        r3 = ranges[3]
        cv3 = [None]
        # pre1: contributors complete by t28 (pair 0); pre2: bins 384-479
        # at t30; fin: bins 480-511 after the last token tile.
        r3_pre1 = [c for c in r3["cons"] if max(seg_tiles[c]) <= 28]
        r3_fin = [c for c in r3["cons"] if c[1] == 96]
        r3_pre2 = [c for c in r3["cons"]
                   if c not in r3_pre1 and c not in r3_fin]
        t_pre1 = 28
        t_pre2 = max(max(seg_tiles[c]) for c in r3_pre2)

        jobs_at = {}
        for r in ranges[:3]:
            jobs_at.setdefault(r["trigger"], []).append(("full", r))
        jobs_at.setdefault(t_pre1, []).append(("pre1", r3))
        jobs_at.setdefault(t_pre2, []).append(("pre2", r3))

        def run_job(kind, r):
            if kind == "full":
                job_range(r, warm=3 if r["ri"] == 2 else 0)
            elif kind == "pre1":
                cv3[0] = ps_f.tile([P, 512], F32, tag="fld", name="cv3")
                conv_mms(cv3[0], r, r3_pre1, True, False, single_start=True)
            else:
                conv_mms(cv3[0], r, r3_pre2, False, False)

        def kwslices(j):
            return kw_sb[:, 2 * j:2 * j + 2, :]

        ksq_tiles = {}

        def emit_k(tk):
            x8 = x8_pairs[tk // 2][:, tk % 2]
            kps = ps_kv.tile([P, CLOC], F32, tag="kv", name="kps")
            for j in range(KT // 2):
                nc.tensor.matmul(kps[:], x8[:, 2 * j:2 * j + 2, :],
                                 kwslices(j),
                                 perf_mode=mybir.MatmulPerfMode.DoubleRow,
                                 start=(j == 0),
                                 stop=(j == KT // 2 - 1 and not with_kb))
            if with_kb:
                nc.tensor.matmul(kps[:], ones_sb[:], kb_sb[:], start=False,
                                 stop=True)
            # square immediately: frees the PSUM slot early and decouples
            # the ||k|| chain from the k/v PE cadence
            ksq = smpool.tile([P, CLOC], F32, tag="ksq", bufs=LEAD + 2)
            nc.scalar.activation(ksq[:], kps[:],
                                 mybir.ActivationFunctionType.Square)
            ksq_tiles[tk] = ksq

        # k prologue; a couple of dependency-free fillers between tiles
        # bridge the x8/kw DMA ramp without delaying anything that has
        # already landed by more than their own duration.
        for tk in range(LEAD):
            emit_k(tk)
            if tk in (1, 3):
                wp = ps_m.tile([P, CLOC], F32, tag="mid", name="warm")
                for i in range(2):
                    nc.tensor.matmul(wp[:], p80[:, 0, 0, :],
                                     p80[:, 0, 0:4, :]
                                     .rearrange("p kt c -> p (kt c)"),
                                     start=True, stop=True)
            if tk % 2 == 1:
                x8_pairs.pop(tk // 2, None)

        qrot = [nc.sync, nc.scalar, nc.gpsimd]
        qi = [0]

        def rot_dma(dst, src):
            qrot[qi[0] % 3].dma_start(dst, src)
            qi[0] += 1

        pending = None
        for t in range(TT):
            tk = t + LEAD

            # prefetch x8 pair for tile tk.. and xb pair for t+2..
            for tp8 in ((t + LEAD) // 2, (t + LEAD + 2) // 2):
                if tp8 < TPAIRS and tp8 not in x8_pairs:
                    rot_dma(*x8p(tp8))
            tpb = (t + 2) // 2
            if tpb < TPAIRS and tpb not in xb_pairs:
                rot_dma(*xbp(tpb))
            if t == 6:
                nc.scalar.dma_start(ct_sb[:], ctt.rearrange(
                    "p (gt g) -> p gt g", gt=GT))
            if t == 10:
                nc.scalar.dma_start(ow_sb[:], owt.rearrange(
                    "p (gt e) -> p gt e", gt=GT))
            if t == 10:
                nc.gpsimd.dma_start(sm_sb[2][:],
                                    smp[2].rearrange("p (i c) -> p i c", c=P))
            if t == 18:
                nc.gpsimd.dma_start(sm_sb[3][:],
                                    smp[3].rearrange("p (i c) -> p i c", c=P))

            if tk < TT:
                emit_k(tk)

            xb = xb_pairs[t // 2][:, t % 2]
            vps = ps_kv.tile([P, CLOC], F32, tag="kv", name="vps")
            for kt in range(KT):
                nc.tensor.matmul(vps[:], xb[:, kt, :], vw_sb[:, kt, :],
                                 start=(kt == 0),
                                 stop=(kt == KT - 1 and not with_vb))
            if with_vb:
                nc.tensor.matmul(vps[:], ones_sb[:], vb_sb[:], start=False,
                                 stop=True)
            # release consumed input pairs
            if t % 2 == 1:
                xb_pairs.pop(t // 2, None)
            if tk % 2 == 1:
                x8_pairs.pop(tk // 2, None)

            # scatter of the previous tile (its wv is ready by now)
            if pending is not None:
                emit_scatter(*pending)
                for kind, r in jobs_at.get(pending[0], []):
                    run_job(kind, r)

            # ||k|| per head from the (already squared) k of tile t
            ksq = ksq_tiles.pop(t)
            km2 = smpool.tile([P, HLOC], F32, tag="km2")
            nc.vector.reduce_sum(km2[:], ksq[:].rearrange("p (h d) -> p h d",
                                                          d=HD),
                                 axis=mybir.AxisListType.X)
            km = smpool.tile([P, HLOC], F32, tag="km")
            nc.scalar.sqrt(km[:], km2[:])

            # wv = v * ||k|| -> bf16
            wv = wvpool.tile([P, CLOC], BF16, tag="wv")
            nc.vector.tensor_tensor(
                wv[:].rearrange("p (h d) -> p h d", d=HD),
                vps[:].rearrange("p (h d) -> p h d", d=HD),
                km[:].unsqueeze(2).broadcast_to((P, HLOC, HD)),
                mybir.AluOpType.mult)
            pending = (t, wv)

        emit_scatter(*pending)
        for kind, r in jobs_at.get(pending[0], []):
            run_job(kind, r)

        # tail: final conv segment (bins 480-511), then convT copy + A
        warm_tail(2)
        conv_mms(cv3[0], r3, r3_fin, False, True)
        fin_convT(r3, cv3[0])
        warm_tail(2)
        job_A(r3)

    nc.compile()
    return nc


_PROGRAM_CACHE = {}
_PLANS_CACHE = {}


def _get_plans():
    if "p" not in _PLANS_CACHE:
        _PLANS_CACHE["p"] = _plans()
    return _PLANS_CACHE["p"]


def _get_program(with_kb, with_vb):
    key = (with_kb, with_vb)
    if key not in _PROGRAM_CACHE:
        _PROGRAM_CACHE[key] = _build_program(with_kb, with_vb, _get_plans())
    return _PROGRAM_CACHE[key]


def kernel(x, q_w, q_b, k_w, k_b, v_w, v_b, out_w, out_b):
    global LAST_RESULT
    x = np.asarray(x, dtype=np.float32)
    k_w = np.asarray(k_w, dtype=np.float32)
    k_b = np.asarray(k_b, dtype=np.float32)
    v_w = np.asarray(v_w, dtype=np.float32)
    v_b = np.asarray(v_b, dtype=np.float32)
    out_w = np.asarray(out_w, dtype=np.float32)
    out_b = np.asarray(out_b, dtype=np.float32)

    with_kb = bool(np.any(k_b))
    with_vb = bool(np.any(v_b))
    nc = _get_program(with_kb, with_vb)
    pl = _get_plans()
    TPAIRS = TT // 2

    # S blocks packed per gt in device order
    smp_h = {}
    for gt in range(GT):
        blocks = [pl["Smat"][t * P:(t + 1) * P, gt * P:(gt + 1) * P]
                  for t in pl["sblocks"][gt]]
        smp_h[gt] = np.ascontiguousarray(
            np.stack(blocks, axis=1).reshape(P, len(blocks) * P)
        ).astype(NP_FP8)

    # 1/KSCALE compensates the x KSCALE on the fp8 k-weights (exact: the
    # bf16 CT values just shift exponent by 5)
    CTm = pl["CTm"] * np.float32(1.0 / KSCALE)
    # ctt[p, gt*G+g] = CTm[gt*128+p, g]
    ctt = np.ascontiguousarray(
        CTm.reshape(GT, P, G).transpose(1, 0, 2).reshape(P, GT * G)
    ).astype(NP_BF16)

    in_maps = []
    for c in range(NCORES):
        b, hg = c // 2, c % 2
        chs = slice(hg * CLOC, (hg + 1) * CLOC)
        # pair layout: x[b] tokens (2tp+j)*128+c, contraction kt*128+p
        xb = x[b].reshape(TPAIRS, 2, P, KT, P).transpose(0, 4, 1, 3, 2) \
            .reshape(TPAIRS * P, 2 * KT * P)
        kwl = (k_w[chs, :].T * np.float32(KSCALE)) \
            .reshape(KT, P, CLOC).transpose(1, 0, 2).reshape(P, KT * CLOC)
        vwl = v_w[chs, :].T.reshape(KT, P, CLOC).transpose(1, 0, 2) \
            .reshape(P, KT * CLOC)
        owl = out_w[:, chs].T.reshape(GT, P, D).transpose(1, 0, 2) \
            .reshape(P, GT * D)
        m = {
            "xTt": np.ascontiguousarray(xb).astype(NP_BF16),
            "x8t": np.ascontiguousarray(xb).astype(NP_FP8),
            "kwt": np.ascontiguousarray(kwl).astype(NP_FP8),
            "vwt": np.ascontiguousarray(vwl).astype(NP_BF16),
            "owt": np.ascontiguousarray(owl).astype(NP_BF16),
            "ctt": ctt,
        }
        for gt in range(GT):
            m[f"smp{gt}"] = smp_h[gt]
        if with_kb:
            m["kb"] = np.ascontiguousarray(
                k_b[chs][None, :] * np.float32(KSCALE)).astype(NP_BF16)
        if with_vb:
            m["vb"] = np.ascontiguousarray(v_b[chs][None, :]).astype(NP_BF16)
        if with_kb or with_vb:
            m["ones"] = np.ones((1, P), dtype=NP_BF16)
        in_maps.append(m)

    res = run_bass_kernel_spmd(nc, in_maps, core_ids=list(range(NCORES)),
                               trace=TRACE)
    LAST_RESULT = res

    idx = pl["idx"]
    out = np.empty((B, N, D), dtype=np.float32)
    for b in range(B):
        # unshard: sum the two head-group partials of A, then replicate
        # bin rows out to tokens (pure gather) and add the output bias.
        A = np.zeros((G, D), dtype=np.float32)
        for part in (res.results[2 * b]["aout"], res.results[2 * b + 1]["aout"]):
            for r in pl["ranges"]:
                ri, glo, ghi = r["ri"], r["glo"], r["ghi"]
                A[glo:ghi] += part[ri * P:ri * P + (ghi - glo)]
        out[b] = A[idx]
        out[b] += out_b[None, :]
    return out


# revision 36
# speedup vs baseline: 1.0575x; 1.0575x over previous
"""Trainium2 Bass kernel for nn_CausalFieldAttention (v3).

Shapes (hardcoded): B=4, N=4096, D=1024, H=16, hd=64, G=512, sigma=3.

Reference computation (q-projection is computed but unused -> skipped):
    k  = x @ k_w.T + k_b                      (B,N,D) -> heads (B,H,N,hd)
    v  = x @ v_w.T + v_b
    wv = v * ||k||_head
    field = segment_sum(wv, field_idx, G)     scatter tokens -> G bins
    conv  = circular_conv(field, causal_ker)  (exact circulant)
    y  = conv[field_idx]                      gather bins -> tokens
    out = y @ out_w.T + out_b
Device computes A = conv @ ow at bin granularity; host replicates bin
rows to tokens (pure gather) and sums the two head-group partials.

Device strategy: 8 cores = 4 batches x 2 head-groups (512 channels each).
v3 changes vs v2 (145.7us):
  - Scatter at pair (128-bin) granularity: one [128tok->128bin, 512] MM
    per (tile, gt) instead of per 64-bin half => ~25% fewer scatter MMs,
    full 128-wide PE columns.  The last bin-tile (gt3) keeps fine
    granularity in three column segments (bins 384-447 / 448-479 /
    480-511) so the structural tail is only the final 32-bin segment.
  - Conv contracts K=128 over full pairs (K=64/K=32 for gt3 segments)
    and uses four uniform W=128 g-ranges => 44 cheap N=128 conv MMs and
    32 A-proj MMs (vs 72 + 40).
  - Tail range [128,256) accumulates its conv directly in PSUM across
    t28/t30/t31 partial triggers: no SBUF pre-accumulate + DVE add, and
    the post-last-token work is 4 K=32 conv MMs + 8 A MMs + 0.5MB DMA.
  - DMA descriptor payloads: x8/xb loaded as 2-tile pairs (2/4KB rows),
    kw/vw/ct/ow as whole tensors (4-8KB rows), S-blocks host-packed per
    gt (2.25KB rows).  Startup DMAs are deadline-ordered across the
    three queues; dependency-free warm-up MMs on the first-landed x8
    pair pull the PE out of its cold HAM state during the DMA ramp.
"""

import os
import sys
from contextlib import ExitStack

import numpy as np

for _p in ("/opt/trn_rl_repo", "/root/.axon_site/_ro/trn_rl_repo"):
    if os.path.isdir(_p) and _p not in sys.path:
        sys.path.append(_p)

import concourse.bacc as bacc
import concourse.mybir as mybir
import concourse.tile as tile
from concourse.bass_utils import run_bass_kernel_spmd

B, N, D = 4, 4096, 1024
H, HD, G = 16, 64, 512
SIGMA = 3.0
P = 128
KT = D // P          # 8 contraction tiles over D
TT = N // P          # 32 token tiles
GT = G // P          # 4 bin pair-tiles
CLOC = 512           # channels per core (8 heads)
HLOC = CLOC // HD    # 8 heads per core
ECH = D // 512       # 2 chunks of out-channels for 512-wide psum
NCORES = 8
LEAD = 4             # k-projection runs this many tiles ahead of v

F32 = mybir.dt.float32
BF16 = mybir.dt.bfloat16
FP8 = mybir.dt.float8e4
NP_BF16 = mybir.dt.np(BF16)
NP_FP8 = mybir.dt.np(FP8)
KSCALE = 32.0   # k-weights are scaled x32 into fp8's normal range; the
                # resulting 32x on ||k|| is compensated exactly (power of
                # two) by scaling the conv matrix by 1/32.
VSCALE = 32.0   # v-weights likewise x32 (the first 256 contraction dims
                # run in fp8 DoubleRow); compensated in the conv matrix.
VFP8 = 2        # number of leading v k-tiles done in fp8 DoubleRow

# gt3 column segments (pair-partition coords; bin = 384 + p)
SEGS = ((0, 96), (96, 128))

# set by test harness to capture a profile; kernel() stores results here
TRACE = False
LAST_RESULT = None


def _field_idx():
    # exactly mirrors the reference (fp32 div then mul, trunc, clip)
    pos = np.arange(N, dtype=np.float32) / np.float32(N - 1) * np.float32(G - 1)
    return np.clip(pos.astype(np.int32), 0, G - 1)


def _causal_kernel():
    i = np.arange(G)
    dist = np.abs(i - G // 2)
    ker = np.where(i >= G // 2, 0.0, np.exp(-dist / SIGMA)).astype(np.float32)
    ker = ker / (ker.sum() + 1e-8)
    return ker


def _plans():
    idx = _field_idx()
    ker = _causal_kernel()
    gg = (np.arange(G)[None, :] - np.arange(G)[:, None]) % G
    CTm = ker[gg].astype(np.float32)      # CTm[f, g] = ker[(g-f)%G]

    Smat = np.zeros((N, G), np.float32)
    Smat[np.arange(N), idx] = 1.0

    counts = np.bincount(idx, minlength=G)           # tokens per bin
    tok_start = np.concatenate([[0], np.cumsum(counts)])

    # kernel support: ker[m] > 1e-12 for m in [mlo, mhi] = [176, 255]
    nz = np.where(ker > 1e-12)[0]
    mlo, mhi = int(nz.min()), int(nz.max())

    # scatter segments: (gt, plo, phi) with bin range [gt*128+plo, gt*128+phi)
    segments = [(gt, 0, P) for gt in range(3)]
    segments += [(3, lo, hi) for lo, hi in SEGS]

    def tiles_touching(b0, b1):
        ts = []
        for t in range(TT):
            bt = idx[t * P:(t + 1) * P]
            if np.any((bt >= b0) & (bt < b1)):
                ts.append(t)
        return ts

    seg_tiles = {}
    for gt, plo, phi in segments:
        seg_tiles[(gt, plo, phi)] = tiles_touching(gt * P + plo, gt * P + phi)

    tile_gts = []
    for t in range(TT):
        bt = idx[t * P:(t + 1) * P]
        tile_gts.append(sorted(set((bt // P).tolist())))

    # ranges: conv[g] needs field bins [(g-mhi)%G, (g-mlo)%G].
    def contributors(glo, ghi):
        need = set()
        for g in range(glo, ghi):
            for m in range(mlo, mhi + 1):
                need.add((g - m) % G)
        out = []
        for gt, plo, phi in segments:
            if any((gt * P + p) in need for p in range(plo, phi)):
                out.append((gt, plo, phi))
        return out

    ranges = []
    for ri, glo in enumerate((256, 384, 0, 128)):
        ghi = glo + P
        cons = contributors(glo, ghi)
        trig = max(max(seg_tiles[c]) for c in cons)
        chunks = []
        b = glo
        while b < ghi:
            c = int(counts[b])
            nb = 1
            while b + nb < ghi and int(counts[b + nb]) == c:
                nb += 1
            chunks.append((int(tok_start[b]), b, nb, c))
            b += nb
        ranges.append({"ri": ri, "glo": glo, "ghi": ghi, "cons": cons,
                       "trigger": trig, "chunks": chunks})

    # S-block list per gt (host packs these in order)
    sblocks = {gt: [t for t in range(TT) if gt in tile_gts[t]]
               for gt in range(GT)}

    return {
        "idx": idx, "CTm": CTm, "Smat": Smat,
        "segments": segments, "seg_tiles": seg_tiles,
        "tile_gts": tile_gts, "ranges": ranges, "sblocks": sblocks,
        "counts": counts,
    }


def _build_program(with_kb, with_vb, pl):
    tile_gts = pl["tile_gts"]
    seg_tiles = pl["seg_tiles"]
    segments = pl["segments"]
    ranges = pl["ranges"]
    sblocks = pl["sblocks"]
    NR = len(ranges)
    TPAIRS = TT // 2

    nc = bacc.Bacc("TRN2", target_bir_lowering=False, debug=False,
                   num_devices=NCORES)
    # host-permuted layouts; x8/xb pack TWO token tiles per 128-partition
    # block so every DMA row is 2/4KB.
    xTt = nc.dram_tensor("xTt", [TPAIRS * P, 2 * KT * P], BF16,
                         kind="ExternalInput").ap()
    x8t = nc.dram_tensor("x8t", [TPAIRS * P, 2 * KT * P], FP8,
                         kind="ExternalInput").ap()
    kwt = nc.dram_tensor("kwt", [P, KT * CLOC], FP8, kind="ExternalInput").ap()
    vwt = nc.dram_tensor("vwt", [P, KT * CLOC], BF16, kind="ExternalInput").ap()
    vw8t = nc.dram_tensor("vw8t", [P, VFP8 * CLOC], FP8,
                          kind="ExternalInput").ap()
    owt = nc.dram_tensor("owt", [P, GT * D], BF16, kind="ExternalInput").ap()
    ctt = nc.dram_tensor("ctt", [P, GT * G], BF16, kind="ExternalInput").ap()
    smp = {gt: nc.dram_tensor(f"smp{gt}", [P, len(sblocks[gt]) * P], FP8,
                              kind="ExternalInput").ap() for gt in range(GT)}
    kb = nc.dram_tensor("kb", [1, CLOC], BF16, kind="ExternalInput").ap() if with_kb else None
    vb = nc.dram_tensor("vb", [1, CLOC], BF16, kind="ExternalInput").ap() if with_vb else None
    ones_d = (nc.dram_tensor("ones", [1, P], BF16, kind="ExternalInput").ap()
              if (with_kb or with_vb) else None)
    aout = nc.dram_tensor("aout", [NR * P, D], BF16,
                          kind="ExternalOutput").ap()

    with tile.TileContext(nc) as tc, ExitStack() as es:
        cpool = es.enter_context(tc.tile_pool(name="const", bufs=1))

        kw_sb = cpool.tile([P, KT, CLOC], FP8)
        vw_sb = cpool.tile([P, KT, CLOC], BF16)
        vw8_sb = cpool.tile([P, VFP8, CLOC], FP8)
        ow_sb = cpool.tile([P, GT, D], BF16)
        ct_sb = cpool.tile([P, GT, G], BF16)        # [f%128, f//128, g]
        field_sb = cpool.tile([P, GT, CLOC], BF16)  # [f%128, f//128, ch]
        convT_sb = cpool.tile([P, GT, G], BF16)     # [ch%128, ch//128, g]
        A_sb = cpool.tile([P, NR, D], BF16)         # [bin-glo(r), r, e]
        sm_sb = {gt: cpool.tile([P, len(sblocks[gt]), P], FP8,
                                name=f"sm{gt}") for gt in range(GT)}
        if with_kb or with_vb:
            ones_sb = cpool.tile([1, P], BF16)
            nc.sync.dma_start(ones_sb[:], ones_d[:])
        if with_kb:
            kb_sb = cpool.tile([1, CLOC], BF16)
            nc.sync.dma_start(kb_sb[:], kb[:])
        if with_vb:
            vb_sb = cpool.tile([1, CLOC], BF16)
            nc.sync.dma_start(vb_sb[:], vb[:])

        xpool = es.enter_context(tc.tile_pool(name="xin", bufs=3))
        x8pool = es.enter_context(tc.tile_pool(name="x8in", bufs=5))
        smpool = es.enter_context(tc.tile_pool(name="small", bufs=3))
        wvpool = es.enter_context(tc.tile_pool(name="wv", bufs=3))
        # 4-deep k/v ring: v(t+1) reuses the slot freed by square(k(t+3)) on
        # ACT; at depth 3 a momentarily busy ACT stalls the PE v-stream.
        ps_kv = es.enter_context(tc.tile_pool(name="ps_kv", bufs=4, space="PSUM"))
        ps_f = es.enter_context(tc.tile_pool(name="ps_f", bufs=2, space="PSUM"))
        ps_m = es.enter_context(tc.tile_pool(name="ps_m", bufs=2, space="PSUM"))

        eng_flip = [0]

        def flip_copy(dst, src):
            # alternate DVE/ACT for PSUM->SBUF traffic
            if eng_flip[0] % 2 == 0:
                nc.vector.tensor_copy(dst, src)
            else:
                nc.scalar.copy(dst, src)
            eng_flip[0] += 1

        # ---- startup DMA plan: deadline-ordered across three queues ----
        x8_pairs = {}
        xb_pairs = {}

        def x8p(tp):
            t8 = x8pool.tile([P, 2, KT, P], FP8, tag="x8blk", bufs=5,
                             name=f"x8p{tp % 5}")
            x8_pairs[tp] = t8
            return (t8[:], x8t[tp * P:(tp + 1) * P, :]
                    .rearrange("p (j kt c) -> p j kt c", j=2, kt=KT))

        def xbp(tp):
            tb = xpool.tile([P, 2, KT, P], BF16, tag="xblk", bufs=3,
                            name=f"xbp{tp % 3}")
            xb_pairs[tp] = tb
            return (tb[:], xTt[tp * P:(tp + 1) * P, :]
                    .rearrange("p (j kt c) -> p j kt c", j=2, kt=KT))

        # first x8/xb pairs land as two per-tile chunks so the earliest
        # matmuls gate on the minimum bytes; kw/vw land in halves.
        x8p(0)
        xbp(0)
        p80 = x8_pairs[0]
        pb0 = xb_pairs[0]
        x8r0 = x8t[0:P, :].rearrange("p (j kt c) -> p j kt c", j=2, kt=KT)
        xbr0 = xTt[0:P, :].rearrange("p (j kt c) -> p j kt c", j=2, kt=KT)
        kwr = kwt.rearrange("p (kt c) -> p kt c", kt=KT)
        vwr = vwt.rearrange("p (kt c) -> p kt c", kt=KT)
        plan = {
            nc.sync: [(p80[:, 0], x8r0[:, 0]),
                      (p80[:, 1], x8r0[:, 1]),
                      x8p(1),
                      (vw_sb[:, 2:5], vwr[:, 2:5])],
            nc.scalar: [(kw_sb[:, 0:4], kwr[:, 0:4]),
                        (kw_sb[:, 4:8], kwr[:, 4:8]),
                        x8p(2),
                        (sm_sb[0][:], smp[0].rearrange("p (i c) -> p i c", c=P))],
            nc.gpsimd: [(vw8_sb[:], vw8t.rearrange("p (kt c) -> p kt c",
                                                     kt=VFP8)),
                        (pb0[:, 0], xbr0[:, 0]),
                        (pb0[:, 1], xbr0[:, 1]),
                        (vw_sb[:, 5:8], vwr[:, 5:8]),
                        (sm_sb[1][:], smp[1].rearrange("p (i c) -> p i c", c=P))],
        }
        for eng, items in plan.items():
            for dst, srcap in items:
                eng.dma_start(dst, srcap)

        # warm-up: dependency-free matmuls on the first x8 chunk bridge the
        # DMA ramp and un-throttle the PE's HAM clock before real work.
        warm_ps = ps_m.tile([P, CLOC], F32, tag="mid", name="warm")
        for i in range(6):
            nc.tensor.matmul(warm_ps[:], x8_pairs[0][:, 0, 0, :],
                             x8_pairs[0][:, 0, 0:4, :]
                             .rearrange("p kt c -> p (kt c)"),
                             start=True, stop=True)

        field_ps = {}
        seg_open = {}

        def emit_scatter(t, wv):
            for gt in tile_gts[t]:
                if gt not in field_ps:
                    field_ps[gt] = ps_f.tile([P, CLOC], F32, tag="fld",
                                             name=f"fld{gt}")
                si = sblocks[gt].index(t)
                st = sm_sb[gt][:, si, :]
                if gt < 3:
                    tts = seg_tiles[(gt, 0, P)]
                    nc.tensor.matmul(field_ps[gt][:], st, wv[:],
                                     start=(t == tts[0]), stop=(t == tts[-1]))
                    if t == tts[-1]:
                        flip_copy(field_sb[:, gt, :], field_ps[gt][:])
                        del field_ps[gt]
                else:
                    for plo, phi in SEGS:
                        tts = seg_tiles[(3, plo, phi)]
                        if t not in tts:
                            continue
                        kw = {}
                        if plo > 0:
                            kw["tile_position"] = (0, plo)
                        nc.tensor.matmul(field_ps[3][plo:phi, :],
                                         st[:, plo:phi], wv[:],
                                         start=(t == tts[0]),
                                         stop=(t == tts[-1]), **kw)
                        if t == tts[-1]:
                            flip_copy(field_sb[plo:phi, 3, :],
                                      field_ps[3][plo:phi, :])
                            seg_open[(plo, phi)] = True
                            if len(seg_open) == len(SEGS):
                                del field_ps[3]

        def conv_mms(cv, r, cons, first, last, single_start=False):
            # cv[ch, ct*W + (g-glo)] accumulated over contributor segments.
            # start=True clears has_written for the WHOLE bank, so a
            # staggered (multi-burst) accumulation must issue it exactly
            # once: later first-writes overwrite naturally via
            # has_written=0, later repeat-writes accumulate.
            glo, ghi = r["glo"], r["ghi"]
            W = ghi - glo
            for ct in range(GT):
                for j, (gt, plo, phi) in enumerate(cons):
                    kw = {}
                    if plo in (64, 96):
                        kw["tile_position"] = (plo, 0)
                    if single_start:
                        st = first and ct == 0 and j == 0
                    else:
                        st = first and j == 0
                    nc.tensor.matmul(
                        cv[:, ct * W:(ct + 1) * W],
                        field_sb[plo:phi, gt, ct * P:(ct + 1) * P],
                        ct_sb[plo:phi, gt, glo:ghi],
                        start=st,
                        stop=(last and j == len(cons) - 1), **kw)

        def job_A(r):
            ri, glo, ghi = r["ri"], r["glo"], r["ghi"]
            W = ghi - glo
            for ec in range(ECH):
                esl = slice(ec * 512, (ec + 1) * 512)
                pa = ps_m.tile([P, 512], F32, tag="mid")
                for ct in range(GT):
                    nc.tensor.matmul(pa[0:W, :],
                                     convT_sb[:, ct, glo:ghi],
                                     ow_sb[:, ct, esl],
                                     start=(ct == 0), stop=(ct == GT - 1))
                # the tail range drains in half-chunks so the last DMA
                # starts as early as possible
                nch = 2 if ri == 3 else 1
                for c in range(nch):
                    cw = 512 // nch
                    el = slice(ec * 512 + c * cw, ec * 512 + (c + 1) * cw)
                    flip_copy(A_sb[0:W, ri, el], pa[0:W, c * cw:(c + 1) * cw])
                    eng = nc.sync if (ec + c) % 2 == 0 else nc.scalar
                    eng.dma_start(aout[ri * P:ri * P + W, el],
                                  A_sb[0:W, ri, el])

        def fin_convT(r, cv):
            glo, ghi = r["glo"], r["ghi"]
            W = ghi - glo
            for lo, hi in ((0, 2), (2, 4)):
                flip_copy(convT_sb[:, lo:hi, glo:ghi],
                          cv[:, lo * W:hi * W].rearrange("p (ct w) -> p ct w",
                                                         w=W))

        def warm_tail(n):
            # dependency-free matmuls on long-resident operands keep the
            # PE's HAM clock at full rate across tail copy-latency bubbles
            wt = ps_kv.tile([P, CLOC], F32, tag="kv", name="warm")
            for i in range(n):
                nc.tensor.matmul(wt[:], field_sb[:, 0, 0:P],
                                 ct_sb[:, 0, 0:CLOC],
                                 start=(i == 0), stop=(i == n - 1))

        def job_range(r, warm=0):
            cv = ps_m.tile([P, 512], F32, tag="mid")
            conv_mms(cv, r, r["cons"], True, True)
            fin_convT(r, cv)
            if warm:
                warm_tail(warm)
            job_A(r)

        # tail range R3 [128,256): conv accumulates in a parked PSUM bank
        # across its staggered contributor triggers.
        r3 = ranges[3]
        cv3 = [None]
        # pre1: contributors complete by t28 (pair 0); pre2: bins 384-479
        # at t30; fin: bins 480-511 after the last token tile.
        r3_pre1 = [c for c in r3["cons"] if max(seg_tiles[c]) <= 28]
        r3_fin = [c for c in r3["cons"] if c[1] == 96]
        r3_pre2 = [c for c in r3["cons"]
                   if c not in r3_pre1 and c not in r3_fin]
        t_pre1 = 28
        t_pre2 = max(max(seg_tiles[c]) for c in r3_pre2)

        jobs_at = {}
        for r in ranges[:3]:
            jobs_at.setdefault(r["trigger"], []).append(("full", r))
        jobs_at.setdefault(t_pre1, []).append(("pre1", r3))
        jobs_at.setdefault(t_pre2, []).append(("pre2", r3))

        def run_job(kind, r):
            if kind == "full":
                job_range(r, warm=3 if r["ri"] == 2 else 0)
            elif kind == "pre1":
                cv3[0] = ps_f.tile([P, 512], F32, tag="fld", name="cv3")
                conv_mms(cv3[0], r, r3_pre1, True, False, single_start=True)
            else:
                conv_mms(cv3[0], r, r3_pre2, False, False)

        def kwslices(j):
            return kw_sb[:, 2 * j:2 * j + 2, :]

        ksq_tiles = {}

        def emit_k(tk):
            x8 = x8_pairs[tk // 2][:, tk % 2]
            kps = ps_kv.tile([P, CLOC], F32, tag="kv", name="kps")
            for j in range(KT // 2):
                nc.tensor.matmul(kps[:], x8[:, 2 * j:2 * j + 2, :],
                                 kwslices(j),
                                 perf_mode=mybir.MatmulPerfMode.DoubleRow,
                                 start=(j == 0),
                                 stop=(j == KT // 2 - 1 and not with_kb))
            if with_kb:
                nc.tensor.matmul(kps[:], ones_sb[:], kb_sb[:], start=False,
                                 stop=True)
            # square immediately: frees the PSUM slot early and decouples
            # the ||k|| chain from the k/v PE cadence
            ksq = smpool.tile([P, CLOC], F32, tag="ksq", bufs=LEAD + 2)
            nc.scalar.activation(ksq[:], kps[:],
                                 mybir.ActivationFunctionType.Square)
            ksq_tiles[tk] = ksq

        # k prologue; a couple of dependency-free fillers between tiles
        # bridge the x8/kw DMA ramp without delaying anything that has
        # already landed by more than their own duration.
        for tk in range(LEAD):
            emit_k(tk)
            if tk in (1, 3):  # noqa: keep fillers
                wp = ps_m.tile([P, CLOC], F32, tag="mid", name="warm")
                for i in range(2):
                    nc.tensor.matmul(wp[:], p80[:, 0, 0, :],
                                     p80[:, 0, 0:4, :]
                                     .rearrange("p kt c -> p (kt c)"),
                                     start=True, stop=True)

        qrot = [nc.sync, nc.scalar, nc.gpsimd]
        qi = [0]

        def rot_dma(dst, src):
            qrot[qi[0] % 3].dma_start(dst, src)
            qi[0] += 1

        pending = None
        for t in range(TT):
            tk = t + LEAD

            # prefetch x8 pair for tile tk.. and xb pair for t+2..
            for tp8 in ((t + LEAD) // 2, (t + LEAD + 2) // 2):
                if tp8 < TPAIRS and tp8 not in x8_pairs:
                    rot_dma(*x8p(tp8))
            tpb = (t + 2) // 2
            if tpb < TPAIRS and tpb not in xb_pairs:
                rot_dma(*xbp(tpb))
            if t == 6:
                nc.scalar.dma_start(ct_sb[:], ctt.rearrange(
                    "p (gt g) -> p gt g", gt=GT))
            if t == 10:
                nc.scalar.dma_start(ow_sb[:], owt.rearrange(
                    "p (gt e) -> p gt e", gt=GT))
            if t == 10:
                nc.gpsimd.dma_start(sm_sb[2][:],
                                    smp[2].rearrange("p (i c) -> p i c", c=P))
            if t == 18:
                nc.gpsimd.dma_start(sm_sb[3][:],
                                    smp[3].rearrange("p (i c) -> p i c", c=P))

            if tk < TT:
                emit_k(tk)

            xb = xb_pairs[t // 2][:, t % 2]
            x8v = x8_pairs[t // 2][:, t % 2]
            vps = ps_kv.tile([P, CLOC], F32, tag="kv", name="vps")
            nc.tensor.matmul(vps[:], x8v[:, 0:VFP8, :], vw8_sb[:],
                             perf_mode=mybir.MatmulPerfMode.DoubleRow,
                             start=True, stop=False)
            for kt in range(VFP8, KT):
                nc.tensor.matmul(vps[:], xb[:, kt, :], vw_sb[:, kt, :],
                                 start=False,
                                 stop=(kt == KT - 1 and not with_vb))
            if with_vb:
                nc.tensor.matmul(vps[:], ones_sb[:], vb_sb[:], start=False,
                                 stop=True)
            # release consumed input pairs (x8 is read by both the
            # k-lead and this tile's v DoubleRow matmul)
            if t % 2 == 1:
                xb_pairs.pop(t // 2, None)
                x8_pairs.pop(t // 2, None)

            # scatter of the previous tile (its wv is ready by now)
            if pending is not None:
                emit_scatter(*pending)
                for kind, r in jobs_at.get(pending[0], []):
                    run_job(kind, r)

            # ||k|| per head from the (already squared) k of tile t
            ksq = ksq_tiles.pop(t)
            km2 = smpool.tile([P, HLOC], F32, tag="km2")
            nc.vector.reduce_sum(km2[:], ksq[:].rearrange("p (h d) -> p h d",
                                                          d=HD),
                                 axis=mybir.AxisListType.X)
            km = smpool.tile([P, HLOC], F32, tag="km")
            nc.scalar.sqrt(km[:], km2[:])

            # wv = v * ||k|| -> bf16
            wv = wvpool.tile([P, CLOC], BF16, tag="wv")
            nc.vector.tensor_tensor(
                wv[:].rearrange("p (h d) -> p h d", d=HD),
                vps[:].rearrange("p (h d) -> p h d", d=HD),
                km[:].unsqueeze(2).broadcast_to((P, HLOC, HD)),
                mybir.AluOpType.mult)
            pending = (t, wv)

        emit_scatter(*pending)
        for kind, r in jobs_at.get(pending[0], []):
            run_job(kind, r)

        # tail: final conv segment (bins 480-511), then convT copy + A
        warm_tail(2)
        conv_mms(cv3[0], r3, r3_fin, False, True)
        fin_convT(r3, cv3[0])
        warm_tail(2)
        job_A(r3)

    nc.compile()
    return nc


_PROGRAM_CACHE = {}
_PLANS_CACHE = {}


def _get_plans():
    if "p" not in _PLANS_CACHE:
        _PLANS_CACHE["p"] = _plans()
    return _PLANS_CACHE["p"]


def _get_program(with_kb, with_vb):
    key = (with_kb, with_vb)
    if key not in _PROGRAM_CACHE:
        _PROGRAM_CACHE[key] = _build_program(with_kb, with_vb, _get_plans())
    return _PROGRAM_CACHE[key]


def kernel(x, q_w, q_b, k_w, k_b, v_w, v_b, out_w, out_b):
    global LAST_RESULT
    x = np.asarray(x, dtype=np.float32)
    k_w = np.asarray(k_w, dtype=np.float32)
    k_b = np.asarray(k_b, dtype=np.float32)
    v_w = np.asarray(v_w, dtype=np.float32)
    v_b = np.asarray(v_b, dtype=np.float32)
    out_w = np.asarray(out_w, dtype=np.float32)
    out_b = np.asarray(out_b, dtype=np.float32)

    with_kb = bool(np.any(k_b))
    with_vb = bool(np.any(v_b))
    nc = _get_program(with_kb, with_vb)
    pl = _get_plans()
    TPAIRS = TT // 2

    # S blocks packed per gt in device order
    smp_h = {}
    for gt in range(GT):
        blocks = [pl["Smat"][t * P:(t + 1) * P, gt * P:(gt + 1) * P]
                  for t in pl["sblocks"][gt]]
        smp_h[gt] = np.ascontiguousarray(
            np.stack(blocks, axis=1).reshape(P, len(blocks) * P)
        ).astype(NP_FP8)

    # 1/KSCALE compensates the x KSCALE on the fp8 k-weights (exact: the
    # bf16 CT values just shift exponent by 5)
    CTm = pl["CTm"] * np.float32(1.0 / (KSCALE * VSCALE))
    # ctt[p, gt*G+g] = CTm[gt*128+p, g]
    ctt = np.ascontiguousarray(
        CTm.reshape(GT, P, G).transpose(1, 0, 2).reshape(P, GT * G)
    ).astype(NP_BF16)

    in_maps = []
    for c in range(NCORES):
        b, hg = c // 2, c % 2
        chs = slice(hg * CLOC, (hg + 1) * CLOC)
        # pair layout: x[b] tokens (2tp+j)*128+c, contraction kt*128+p
        xb = x[b].reshape(TPAIRS, 2, P, KT, P).transpose(0, 4, 1, 3, 2) \
            .reshape(TPAIRS * P, 2 * KT * P)
        kwl = (k_w[chs, :].T * np.float32(KSCALE)) \
            .reshape(KT, P, CLOC).transpose(1, 0, 2).reshape(P, KT * CLOC)
        vwl = (v_w[chs, :].T * np.float32(VSCALE)) \
            .reshape(KT, P, CLOC).transpose(1, 0, 2).reshape(P, KT * CLOC)
        vw8l = (v_w[chs, :].T[0:VFP8 * P] * np.float32(VSCALE)) \
            .reshape(VFP8, P, CLOC).transpose(1, 0, 2).reshape(P, VFP8 * CLOC)
        owl = out_w[:, chs].T.reshape(GT, P, D).transpose(1, 0, 2) \
            .reshape(P, GT * D)
        m = {
            "xTt": np.ascontiguousarray(xb).astype(NP_BF16),
            "x8t": np.ascontiguousarray(xb).astype(NP_FP8),
            "kwt": np.ascontiguousarray(kwl).astype(NP_FP8),
            "vwt": np.ascontiguousarray(vwl).astype(NP_BF16),
            "vw8t": np.ascontiguousarray(vw8l).astype(NP_FP8),
            "owt": np.ascontiguousarray(owl).astype(NP_BF16),
            "ctt": ctt,
        }
        for gt in range(GT):
            m[f"smp{gt}"] = smp_h[gt]
        if with_kb:
            m["kb"] = np.ascontiguousarray(
                k_b[chs][None, :] * np.float32(KSCALE)).astype(NP_BF16)
        if with_vb:
            m["vb"] = np.ascontiguousarray(
                v_b[chs][None, :] * np.float32(VSCALE)).astype(NP_BF16)
        if with_kb or with_vb:
            m["ones"] = np.ones((1, P), dtype=NP_BF16)
        in_maps.append(m)

    res = run_bass_kernel_spmd(nc, in_maps, core_ids=list(range(NCORES)),
                               trace=TRACE)
    LAST_RESULT = res

    idx = pl["idx"]
    out = np.empty((B, N, D), dtype=np.float32)
    for b in range(B):
        # unshard: sum the two head-group partials of A, then replicate
        # bin rows out to tokens (pure gather) and add the output bias.
        A = np.zeros((G, D), dtype=np.float32)
        for part in (res.results[2 * b]["aout"], res.results[2 * b + 1]["aout"]):
            for r in pl["ranges"]:
                ri, glo, ghi = r["ri"], r["glo"], r["ghi"]
                A[glo:ghi] += part[ri * P:ri * P + (ghi - glo)]
        out[b] = A[idx]
        out[b] += out_b[None, :]
    return out


# revision 37
# speedup vs baseline: 1.0766x; 1.0181x over previous
"""Trainium2 Bass kernel for nn_CausalFieldAttention (v3).

Shapes (hardcoded): B=4, N=4096, D=1024, H=16, hd=64, G=512, sigma=3.

Reference computation (q-projection is computed but unused -> skipped):
    k  = x @ k_w.T + k_b                      (B,N,D) -> heads (B,H,N,hd)
    v  = x @ v_w.T + v_b
    wv = v * ||k||_head
    field = segment_sum(wv, field_idx, G)     scatter tokens -> G bins
    conv  = circular_conv(field, causal_ker)  (exact circulant)
    y  = conv[field_idx]                      gather bins -> tokens
    out = y @ out_w.T + out_b
Device computes A = conv @ ow at bin granularity; host replicates bin
rows to tokens (pure gather) and sums the two head-group partials.

Device strategy: 8 cores = 4 batches x 2 head-groups (512 channels each).
v3 changes vs v2 (145.7us):
  - Scatter at pair (128-bin) granularity: one [128tok->128bin, 512] MM
    per (tile, gt) instead of per 64-bin half => ~25% fewer scatter MMs,
    full 128-wide PE columns.  The last bin-tile (gt3) keeps fine
    granularity in three column segments (bins 384-447 / 448-479 /
    480-511) so the structural tail is only the final 32-bin segment.
  - Conv contracts K=128 over full pairs (K=64/K=32 for gt3 segments)
    and uses four uniform W=128 g-ranges => 44 cheap N=128 conv MMs and
    32 A-proj MMs (vs 72 + 40).
  - Tail range [128,256) accumulates its conv directly in PSUM across
    t28/t30/t31 partial triggers: no SBUF pre-accumulate + DVE add, and
    the post-last-token work is 4 K=32 conv MMs + 8 A MMs + 0.5MB DMA.
  - DMA descriptor payloads: x8/xb loaded as 2-tile pairs (2/4KB rows),
    kw/vw/ct/ow as whole tensors (4-8KB rows), S-blocks host-packed per
    gt (2.25KB rows).  Startup DMAs are deadline-ordered across the
    three queues; dependency-free warm-up MMs on the first-landed x8
    pair pull the PE out of its cold HAM state during the DMA ramp.
"""

import os
import sys
from contextlib import ExitStack

import numpy as np

for _p in ("/opt/trn_rl_repo", "/root/.axon_site/_ro/trn_rl_repo"):
    if os.path.isdir(_p) and _p not in sys.path:
        sys.path.append(_p)

import concourse.bacc as bacc
import concourse.mybir as mybir
import concourse.tile as tile
from concourse.bass_utils import run_bass_kernel_spmd

B, N, D = 4, 4096, 1024
H, HD, G = 16, 64, 512
SIGMA = 3.0
P = 128
KT = D // P          # 8 contraction tiles over D
TT = N // P          # 32 token tiles
GT = G // P          # 4 bin pair-tiles
CLOC = 512           # channels per core (8 heads)
HLOC = CLOC // HD    # 8 heads per core
ECH = D // 512       # 2 chunks of out-channels for 512-wide psum
NCORES = 8
LEAD = 4             # k-projection runs this many tiles ahead of v

F32 = mybir.dt.float32
BF16 = mybir.dt.bfloat16
FP8 = mybir.dt.float8e4
NP_BF16 = mybir.dt.np(BF16)
NP_FP8 = mybir.dt.np(FP8)
KSCALE = 32.0   # k-weights are scaled x32 into fp8's normal range; the
                # resulting 32x on ||k|| is compensated exactly (power of
                # two) by scaling the conv matrix by 1/32.
VSCALE = 32.0   # v-weights likewise x32 (the first 256 contraction dims
                # run in fp8 DoubleRow); compensated in the conv matrix.
VFP8 = 2        # number of leading v k-tiles done in fp8 DoubleRow

# gt3 column segments (pair-partition coords; bin = 384 + p)
SEGS = ((0, 96), (96, 128))

# set by test harness to capture a profile; kernel() stores results here
TRACE = False
LAST_RESULT = None


def _field_idx():
    # exactly mirrors the reference (fp32 div then mul, trunc, clip)
    pos = np.arange(N, dtype=np.float32) / np.float32(N - 1) * np.float32(G - 1)
    return np.clip(pos.astype(np.int32), 0, G - 1)


def _causal_kernel():
    i = np.arange(G)
    dist = np.abs(i - G // 2)
    ker = np.where(i >= G // 2, 0.0, np.exp(-dist / SIGMA)).astype(np.float32)
    ker = ker / (ker.sum() + 1e-8)
    return ker


def _plans():
    idx = _field_idx()
    ker = _causal_kernel()
    gg = (np.arange(G)[None, :] - np.arange(G)[:, None]) % G
    CTm = ker[gg].astype(np.float32)      # CTm[f, g] = ker[(g-f)%G]

    Smat = np.zeros((N, G), np.float32)
    Smat[np.arange(N), idx] = 1.0

    counts = np.bincount(idx, minlength=G)           # tokens per bin
    tok_start = np.concatenate([[0], np.cumsum(counts)])

    # kernel support: ker[m] > 1e-12 for m in [mlo, mhi] = [176, 255]
    nz = np.where(ker > 1e-12)[0]
    mlo, mhi = int(nz.min()), int(nz.max())

    # scatter segments: (gt, plo, phi) with bin range [gt*128+plo, gt*128+phi)
    segments = [(gt, 0, P) for gt in range(3)]
    segments += [(3, lo, hi) for lo, hi in SEGS]

    def tiles_touching(b0, b1):
        ts = []
        for t in range(TT):
            bt = idx[t * P:(t + 1) * P]
            if np.any((bt >= b0) & (bt < b1)):
                ts.append(t)
        return ts

    seg_tiles = {}
    for gt, plo, phi in segments:
        seg_tiles[(gt, plo, phi)] = tiles_touching(gt * P + plo, gt * P + phi)

    tile_gts = []
    for t in range(TT):
        bt = idx[t * P:(t + 1) * P]
        tile_gts.append(sorted(set((bt // P).tolist())))

    # ranges: conv[g] needs field bins [(g-mhi)%G, (g-mlo)%G].
    def contributors(glo, ghi):
        need = set()
        for g in range(glo, ghi):
            for m in range(mlo, mhi + 1):
                need.add((g - m) % G)
        out = []
        for gt, plo, phi in segments:
            if any((gt * P + p) in need for p in range(plo, phi)):
                out.append((gt, plo, phi))
        return out

    ranges = []
    for ri, glo in enumerate((256, 384, 0, 128)):
        ghi = glo + P
        cons = contributors(glo, ghi)
        trig = max(max(seg_tiles[c]) for c in cons)
        chunks = []
        b = glo
        while b < ghi:
            c = int(counts[b])
            nb = 1
            while b + nb < ghi and int(counts[b + nb]) == c:
                nb += 1
            chunks.append((int(tok_start[b]), b, nb, c))
            b += nb
        ranges.append({"ri": ri, "glo": glo, "ghi": ghi, "cons": cons,
                       "trigger": trig, "chunks": chunks})

    # S-block list per gt (host packs these in order)
    sblocks = {gt: [t for t in range(TT) if gt in tile_gts[t]]
               for gt in range(GT)}

    return {
        "idx": idx, "CTm": CTm, "Smat": Smat,
        "segments": segments, "seg_tiles": seg_tiles,
        "tile_gts": tile_gts, "ranges": ranges, "sblocks": sblocks,
        "counts": counts,
    }


def _build_program(with_kb, with_vb, pl):
    tile_gts = pl["tile_gts"]
    seg_tiles = pl["seg_tiles"]
    segments = pl["segments"]
    ranges = pl["ranges"]
    sblocks = pl["sblocks"]
    NR = len(ranges)
    TPAIRS = TT // 2

    nc = bacc.Bacc("TRN2", target_bir_lowering=False, debug=False,
                   num_devices=NCORES)
    # host-permuted layouts; x8/xb pack TWO token tiles per 128-partition
    # block so every DMA row is 2/4KB.
    KTB = KT - VFP8      # bf16 x stream carries only k-tiles VFP8..KT-1
    xTt = nc.dram_tensor("xTt", [TPAIRS * P, 2 * KTB * P], BF16,
                         kind="ExternalInput").ap()
    x8t = nc.dram_tensor("x8t", [TPAIRS * P, 2 * KT * P], FP8,
                         kind="ExternalInput").ap()
    kwt = nc.dram_tensor("kwt", [P, KT * CLOC], FP8, kind="ExternalInput").ap()
    vwt = nc.dram_tensor("vwt", [P, KT * CLOC], BF16, kind="ExternalInput").ap()
    vw8t = nc.dram_tensor("vw8t", [P, VFP8 * CLOC], FP8,
                          kind="ExternalInput").ap()
    owt = nc.dram_tensor("owt", [P, GT * D], BF16, kind="ExternalInput").ap()
    ctt = nc.dram_tensor("ctt", [P, GT * G], BF16, kind="ExternalInput").ap()
    smp = {gt: nc.dram_tensor(f"smp{gt}", [P, len(sblocks[gt]) * P], FP8,
                              kind="ExternalInput").ap() for gt in range(GT)}
    kb = nc.dram_tensor("kb", [1, CLOC], BF16, kind="ExternalInput").ap() if with_kb else None
    vb = nc.dram_tensor("vb", [1, CLOC], BF16, kind="ExternalInput").ap() if with_vb else None
    ones_d = (nc.dram_tensor("ones", [1, P], BF16, kind="ExternalInput").ap()
              if (with_kb or with_vb) else None)
    aout = nc.dram_tensor("aout", [NR * P, D], BF16,
                          kind="ExternalOutput").ap()

    with tile.TileContext(nc) as tc, ExitStack() as es:
        cpool = es.enter_context(tc.tile_pool(name="const", bufs=1))

        kw_sb = cpool.tile([P, KT, CLOC], FP8)
        vw_sb = cpool.tile([P, KT, CLOC], BF16)
        vw8_sb = cpool.tile([P, VFP8, CLOC], FP8)
        ow_sb = cpool.tile([P, GT, D], BF16)
        ct_sb = cpool.tile([P, GT, G], BF16)        # [f%128, f//128, g]
        field_sb = cpool.tile([P, GT, CLOC], BF16)  # [f%128, f//128, ch]
        convT_sb = cpool.tile([P, GT, G], BF16)     # [ch%128, ch//128, g]
        A_sb = cpool.tile([P, NR, D], BF16)         # [bin-glo(r), r, e]
        sm_sb = {gt: cpool.tile([P, len(sblocks[gt]), P], FP8,
                                name=f"sm{gt}") for gt in range(GT)}
        if with_kb or with_vb:
            ones_sb = cpool.tile([1, P], BF16)
            nc.sync.dma_start(ones_sb[:], ones_d[:])
        if with_kb:
            kb_sb = cpool.tile([1, CLOC], BF16)
            nc.sync.dma_start(kb_sb[:], kb[:])
        if with_vb:
            vb_sb = cpool.tile([1, CLOC], BF16)
            nc.sync.dma_start(vb_sb[:], vb[:])

        xpool = es.enter_context(tc.tile_pool(name="xin", bufs=3))
        x8pool = es.enter_context(tc.tile_pool(name="x8in", bufs=5))
        smpool = es.enter_context(tc.tile_pool(name="small", bufs=3))
        wvpool = es.enter_context(tc.tile_pool(name="wv", bufs=3))
        # 4-deep k/v ring: v(t+1) reuses the slot freed by square(k(t+3)) on
        # ACT; at depth 3 a momentarily busy ACT stalls the PE v-stream.
        ps_kv = es.enter_context(tc.tile_pool(name="ps_kv", bufs=4, space="PSUM"))
        ps_f = es.enter_context(tc.tile_pool(name="ps_f", bufs=2, space="PSUM"))
        ps_m = es.enter_context(tc.tile_pool(name="ps_m", bufs=2, space="PSUM"))

        eng_flip = [0]

        def flip_copy(dst, src):
            # alternate DVE/ACT for PSUM->SBUF traffic
            if eng_flip[0] % 2 == 0:
                nc.vector.tensor_copy(dst, src)
            else:
                nc.scalar.copy(dst, src)
            eng_flip[0] += 1

        # ---- startup DMA plan: deadline-ordered across three queues ----
        x8_pairs = {}
        xb_pairs = {}

        def x8p(tp):
            t8 = x8pool.tile([P, 2, KT, P], FP8, tag="x8blk", bufs=5,
                             name=f"x8p{tp % 5}")
            x8_pairs[tp] = t8
            return (t8[:], x8t[tp * P:(tp + 1) * P, :]
                    .rearrange("p (j kt c) -> p j kt c", j=2, kt=KT))

        def xbp(tp):
            tb = xpool.tile([P, 2, KTB, P], BF16, tag="xblk", bufs=3,
                            name=f"xbp{tp % 3}")
            xb_pairs[tp] = tb
            return (tb[:], xTt[tp * P:(tp + 1) * P, :]
                    .rearrange("p (j kt c) -> p j kt c", j=2, kt=KTB))

        # first x8/xb pairs land as two per-tile chunks so the earliest
        # matmuls gate on the minimum bytes; kw/vw land in halves.
        x8p(0)
        xbp(0)
        p80 = x8_pairs[0]
        pb0 = xb_pairs[0]
        x8r0 = x8t[0:P, :].rearrange("p (j kt c) -> p j kt c", j=2, kt=KT)
        xbr0 = xTt[0:P, :].rearrange("p (j kt c) -> p j kt c", j=2, kt=KTB)
        kwr = kwt.rearrange("p (kt c) -> p kt c", kt=KT)
        vwr = vwt.rearrange("p (kt c) -> p kt c", kt=KT)
        plan = {
            nc.sync: [(p80[:, 0], x8r0[:, 0]),
                      (p80[:, 1], x8r0[:, 1]),
                      x8p(1),
                      (vw_sb[:, 2:5], vwr[:, 2:5])],
            nc.scalar: [(kw_sb[:, 0:4], kwr[:, 0:4]),
                        (kw_sb[:, 4:8], kwr[:, 4:8]),
                        x8p(2),
                        (sm_sb[0][:], smp[0].rearrange("p (i c) -> p i c", c=P))],
            nc.gpsimd: [(vw8_sb[:], vw8t.rearrange("p (kt c) -> p kt c",
                                                     kt=VFP8)),
                        (pb0[:, 0], xbr0[:, 0]),
                        (pb0[:, 1], xbr0[:, 1]),
                        (vw_sb[:, 5:8], vwr[:, 5:8]),
                        (sm_sb[1][:], smp[1].rearrange("p (i c) -> p i c", c=P))],
        }
        for eng, items in plan.items():
            for dst, srcap in items:
                eng.dma_start(dst, srcap)

        # warm-up: dependency-free matmuls on the first x8 chunk bridge the
        # DMA ramp and un-throttle the PE's HAM clock before real work.
        warm_ps = ps_m.tile([P, CLOC], F32, tag="mid", name="warm")
        for i in range(4):
            nc.tensor.matmul(warm_ps[:], x8_pairs[0][:, 0, 0, :],
                             x8_pairs[0][:, 0, 0:4, :]
                             .rearrange("p kt c -> p (kt c)"),
                             start=True, stop=True)

        field_ps = {}
        seg_open = {}

        def emit_scatter(t, wv):
            for gt in tile_gts[t]:
                if gt not in field_ps:
                    field_ps[gt] = ps_f.tile([P, CLOC], F32, tag="fld",
                                             name=f"fld{gt}")
                si = sblocks[gt].index(t)
                st = sm_sb[gt][:, si, :]
                if gt < 3:
                    tts = seg_tiles[(gt, 0, P)]
                    nc.tensor.matmul(field_ps[gt][:], st, wv[:],
                                     start=(t == tts[0]), stop=(t == tts[-1]))
                    if t == tts[-1]:
                        flip_copy(field_sb[:, gt, :], field_ps[gt][:])
                        del field_ps[gt]
                else:
                    for plo, phi in SEGS:
                        tts = seg_tiles[(3, plo, phi)]
                        if t not in tts:
                            continue
                        kw = {}
                        if plo > 0:
                            kw["tile_position"] = (0, plo)
                        nc.tensor.matmul(field_ps[3][plo:phi, :],
                                         st[:, plo:phi], wv[:],
                                         start=(t == tts[0]),
                                         stop=(t == tts[-1]), **kw)
                        if t == tts[-1]:
                            flip_copy(field_sb[plo:phi, 3, :],
                                      field_ps[3][plo:phi, :])
                            seg_open[(plo, phi)] = True
                            if len(seg_open) == len(SEGS):
                                del field_ps[3]

        def conv_mms(cv, r, cons, first, last, single_start=False):
            # cv[ch, ct*W + (g-glo)] accumulated over contributor segments.
            # start=True clears has_written for the WHOLE bank, so a
            # staggered (multi-burst) accumulation must issue it exactly
            # once: later first-writes overwrite naturally via
            # has_written=0, later repeat-writes accumulate.
            glo, ghi = r["glo"], r["ghi"]
            W = ghi - glo
            for ct in range(GT):
                for j, (gt, plo, phi) in enumerate(cons):
                    kw = {}
                    if plo in (64, 96):
                        kw["tile_position"] = (plo, 0)
                    if single_start:
                        st = first and ct == 0 and j == 0
                    else:
                        st = first and j == 0
                    nc.tensor.matmul(
                        cv[:, ct * W:(ct + 1) * W],
                        field_sb[plo:phi, gt, ct * P:(ct + 1) * P],
                        ct_sb[plo:phi, gt, glo:ghi],
                        start=st,
                        stop=(last and j == len(cons) - 1), **kw)

        def job_A(r):
            ri, glo, ghi = r["ri"], r["glo"], r["ghi"]
            W = ghi - glo
            for ec in range(ECH):
                esl = slice(ec * 512, (ec + 1) * 512)
                pa = ps_m.tile([P, 512], F32, tag="mid")
                for ct in range(GT):
                    nc.tensor.matmul(pa[0:W, :],
                                     convT_sb[:, ct, glo:ghi],
                                     ow_sb[:, ct, esl],
                                     start=(ct == 0), stop=(ct == GT - 1))
                # the tail range drains in half-chunks so the last DMA
                # starts as early as possible
                nch = 2 if ri == 3 else 1
                for c in range(nch):
                    cw = 512 // nch
                    el = slice(ec * 512 + c * cw, ec * 512 + (c + 1) * cw)
                    flip_copy(A_sb[0:W, ri, el], pa[0:W, c * cw:(c + 1) * cw])
                    eng = nc.sync if (ec + c) % 2 == 0 else nc.scalar
                    eng.dma_start(aout[ri * P:ri * P + W, el],
                                  A_sb[0:W, ri, el])

        def fin_convT(r, cv):
            glo, ghi = r["glo"], r["ghi"]
            W = ghi - glo
            for lo, hi in ((0, 2), (2, 4)):
                flip_copy(convT_sb[:, lo:hi, glo:ghi],
                          cv[:, lo * W:hi * W].rearrange("p (ct w) -> p ct w",
                                                         w=W))

        def warm_tail(n):
            # dependency-free matmuls on long-resident operands keep the
            # PE's HAM clock at full rate across tail copy-latency bubbles
            wt = ps_kv.tile([P, CLOC], F32, tag="kv", name="warm")
            for i in range(n):
                nc.tensor.matmul(wt[:], field_sb[:, 0, 0:P],
                                 ct_sb[:, 0, 0:CLOC],
                                 start=(i == 0), stop=(i == n - 1))

        def job_range(r, warm=0):
            cv = ps_m.tile([P, 512], F32, tag="mid")
            conv_mms(cv, r, r["cons"], True, True)
            fin_convT(r, cv)
            if warm:
                warm_tail(warm)
            job_A(r)

        # tail range R3 [128,256): conv accumulates in a parked PSUM bank
        # across its staggered contributor triggers.
        r3 = ranges[3]
        cv3 = [None]
        # pre1: contributors complete by t28 (pair 0); pre2: bins 384-479
        # at t30; fin: bins 480-511 after the last token tile.
        r3_pre1 = [c for c in r3["cons"] if max(seg_tiles[c]) <= 28]
        r3_fin = [c for c in r3["cons"] if c[1] == 96]
        r3_pre2 = [c for c in r3["cons"]
                   if c not in r3_pre1 and c not in r3_fin]
        t_pre1 = 28
        t_pre2 = max(max(seg_tiles[c]) for c in r3_pre2)

        jobs_at = {}
        for r in ranges[:3]:
            jobs_at.setdefault(r["trigger"], []).append(("full", r))
        jobs_at.setdefault(t_pre1, []).append(("pre1", r3))
        jobs_at.setdefault(t_pre2, []).append(("pre2", r3))

        def run_job(kind, r):
            if kind == "full":
                job_range(r, warm=3 if r["ri"] == 2 else 0)
            elif kind == "pre1":
                cv3[0] = ps_f.tile([P, 512], F32, tag="fld", name="cv3")
                conv_mms(cv3[0], r, r3_pre1, True, False, single_start=True)
            else:
                conv_mms(cv3[0], r, r3_pre2, False, False)

        def kwslices(j):
            return kw_sb[:, 2 * j:2 * j + 2, :]

        ksq_tiles = {}

        def emit_k(tk):
            x8 = x8_pairs[tk // 2][:, tk % 2]
            kps = ps_kv.tile([P, CLOC], F32, tag="kv", name="kps")
            for j in range(KT // 2):
                nc.tensor.matmul(kps[:], x8[:, 2 * j:2 * j + 2, :],
                                 kwslices(j),
                                 perf_mode=mybir.MatmulPerfMode.DoubleRow,
                                 start=(j == 0),
                                 stop=(j == KT // 2 - 1 and not with_kb))
            if with_kb:
                nc.tensor.matmul(kps[:], ones_sb[:], kb_sb[:], start=False,
                                 stop=True)
            # square immediately: frees the PSUM slot early and decouples
            # the ||k|| chain from the k/v PE cadence
            ksq = smpool.tile([P, CLOC], F32, tag="ksq", bufs=LEAD + 2)
            nc.scalar.activation(ksq[:], kps[:],
                                 mybir.ActivationFunctionType.Square)
            ksq_tiles[tk] = ksq

        # k prologue; a couple of dependency-free fillers between tiles
        # bridge the x8/kw DMA ramp without delaying anything that has
        # already landed by more than their own duration.
        for tk in range(LEAD):
            emit_k(tk)
            if tk in (1, 3):  # noqa: keep fillers
                wp = ps_m.tile([P, CLOC], F32, tag="mid", name="warm")
                for i in range(2):
                    nc.tensor.matmul(wp[:], p80[:, 0, 0, :],
                                     p80[:, 0, 0:4, :]
                                     .rearrange("p kt c -> p (kt c)"),
                                     start=True, stop=True)

        qrot = [nc.sync, nc.scalar, nc.gpsimd]
        qi = [0]

        def rot_dma(dst, src):
            qrot[qi[0] % 3].dma_start(dst, src)
            qi[0] += 1

        pending = None
        for t in range(TT):
            tk = t + LEAD

            # prefetch x8 pair for tile tk.. and xb pair for t+2..
            for tp8 in ((t + LEAD) // 2, (t + LEAD + 2) // 2):
                if tp8 < TPAIRS and tp8 not in x8_pairs:
                    rot_dma(*x8p(tp8))
            tpb = (t + 2) // 2
            if tpb < TPAIRS and tpb not in xb_pairs:
                rot_dma(*xbp(tpb))
            if t == 6:
                nc.scalar.dma_start(ct_sb[:], ctt.rearrange(
                    "p (gt g) -> p gt g", gt=GT))
            if t == 10:
                nc.scalar.dma_start(ow_sb[:], owt.rearrange(
                    "p (gt e) -> p gt e", gt=GT))
            if t == 10:
                nc.gpsimd.dma_start(sm_sb[2][:],
                                    smp[2].rearrange("p (i c) -> p i c", c=P))
            if t == 18:
                nc.gpsimd.dma_start(sm_sb[3][:],
                                    smp[3].rearrange("p (i c) -> p i c", c=P))

            if tk < TT:
                emit_k(tk)

            xb = xb_pairs[t // 2][:, t % 2]
            x8v = x8_pairs[t // 2][:, t % 2]
            vps = ps_kv.tile([P, CLOC], F32, tag="kv", name="vps")
            nc.tensor.matmul(vps[:], x8v[:, 0:VFP8, :], vw8_sb[:],
                             perf_mode=mybir.MatmulPerfMode.DoubleRow,
                             start=True, stop=False)
            for kt in range(VFP8, KT):
                nc.tensor.matmul(vps[:], xb[:, kt - VFP8, :],
                                 vw_sb[:, kt, :],
                                 start=False,
                                 stop=(kt == KT - 1 and not with_vb))
            if with_vb:
                nc.tensor.matmul(vps[:], ones_sb[:], vb_sb[:], start=False,
                                 stop=True)
            # release consumed input pairs (x8 is read by both the
            # k-lead and this tile's v DoubleRow matmul)
            if t % 2 == 1:
                xb_pairs.pop(t // 2, None)
                x8_pairs.pop(t // 2, None)

            # scatter of the previous tile (its wv is ready by now)
            if pending is not None:
                emit_scatter(*pending)
                for kind, r in jobs_at.get(pending[0], []):
                    run_job(kind, r)

            # ||k|| per head from the (already squared) k of tile t
            ksq = ksq_tiles.pop(t)
            km2 = smpool.tile([P, HLOC], F32, tag="km2")
            nc.vector.reduce_sum(km2[:], ksq[:].rearrange("p (h d) -> p h d",
                                                          d=HD),
                                 axis=mybir.AxisListType.X)
            km = smpool.tile([P, HLOC], F32, tag="km")
            nc.scalar.sqrt(km[:], km2[:])

            # wv = v * ||k|| -> bf16
            wv = wvpool.tile([P, CLOC], BF16, tag="wv")
            nc.vector.tensor_tensor(
                wv[:].rearrange("p (h d) -> p h d", d=HD),
                vps[:].rearrange("p (h d) -> p h d", d=HD),
                km[:].unsqueeze(2).broadcast_to((P, HLOC, HD)),
                mybir.AluOpType.mult)
            pending = (t, wv)

        emit_scatter(*pending)
        for kind, r in jobs_at.get(pending[0], []):
            run_job(kind, r)

        # tail: final conv segment (bins 480-511), then convT copy + A
        warm_tail(2)
        conv_mms(cv3[0], r3, r3_fin, False, True)
        fin_convT(r3, cv3[0])
        warm_tail(2)
        job_A(r3)

    nc.compile()
    return nc


_PROGRAM_CACHE = {}
_PLANS_CACHE = {}


def _get_plans():
    if "p" not in _PLANS_CACHE:
        _PLANS_CACHE["p"] = _plans()
    return _PLANS_CACHE["p"]


def _get_program(with_kb, with_vb):
    key = (with_kb, with_vb)
    if key not in _PROGRAM_CACHE:
        _PROGRAM_CACHE[key] = _build_program(with_kb, with_vb, _get_plans())
    return _PROGRAM_CACHE[key]


def kernel(x, q_w, q_b, k_w, k_b, v_w, v_b, out_w, out_b):
    global LAST_RESULT
    x = np.asarray(x, dtype=np.float32)
    k_w = np.asarray(k_w, dtype=np.float32)
    k_b = np.asarray(k_b, dtype=np.float32)
    v_w = np.asarray(v_w, dtype=np.float32)
    v_b = np.asarray(v_b, dtype=np.float32)
    out_w = np.asarray(out_w, dtype=np.float32)
    out_b = np.asarray(out_b, dtype=np.float32)

    with_kb = bool(np.any(k_b))
    with_vb = bool(np.any(v_b))
    nc = _get_program(with_kb, with_vb)
    pl = _get_plans()
    TPAIRS = TT // 2

    # S blocks packed per gt in device order
    smp_h = {}
    for gt in range(GT):
        blocks = [pl["Smat"][t * P:(t + 1) * P, gt * P:(gt + 1) * P]
                  for t in pl["sblocks"][gt]]
        smp_h[gt] = np.ascontiguousarray(
            np.stack(blocks, axis=1).reshape(P, len(blocks) * P)
        ).astype(NP_FP8)

    # 1/KSCALE compensates the x KSCALE on the fp8 k-weights (exact: the
    # bf16 CT values just shift exponent by 5)
    CTm = pl["CTm"] * np.float32(1.0 / (KSCALE * VSCALE))
    # ctt[p, gt*G+g] = CTm[gt*128+p, g]
    ctt = np.ascontiguousarray(
        CTm.reshape(GT, P, G).transpose(1, 0, 2).reshape(P, GT * G)
    ).astype(NP_BF16)

    in_maps = []
    for c in range(NCORES):
        b, hg = c // 2, c % 2
        chs = slice(hg * CLOC, (hg + 1) * CLOC)
        # pair layout: x[b] tokens (2tp+j)*128+c, contraction kt*128+p
        xb = x[b].reshape(TPAIRS, 2, P, KT, P).transpose(0, 4, 1, 3, 2)
        xbb = xb[:, :, :, VFP8:, :].reshape(TPAIRS * P, 2 * (KT - VFP8) * P)
        xb = xb.reshape(TPAIRS * P, 2 * KT * P)
        kwl = (k_w[chs, :].T * np.float32(KSCALE)) \
            .reshape(KT, P, CLOC).transpose(1, 0, 2).reshape(P, KT * CLOC)
        vwl = (v_w[chs, :].T * np.float32(VSCALE)) \
            .reshape(KT, P, CLOC).transpose(1, 0, 2).reshape(P, KT * CLOC)
        vw8l = (v_w[chs, :].T[0:VFP8 * P] * np.float32(VSCALE)) \
            .reshape(VFP8, P, CLOC).transpose(1, 0, 2).reshape(P, VFP8 * CLOC)
        owl = out_w[:, chs].T.reshape(GT, P, D).transpose(1, 0, 2) \
            .reshape(P, GT * D)
        m = {
            "xTt": np.ascontiguousarray(xbb).astype(NP_BF16),
            "x8t": np.ascontiguousarray(xb).astype(NP_FP8),
            "kwt": np.ascontiguousarray(kwl).astype(NP_FP8),
            "vwt": np.ascontiguousarray(vwl).astype(NP_BF16),
            "vw8t": np.ascontiguousarray(vw8l).astype(NP_FP8),
            "owt": np.ascontiguousarray(owl).astype(NP_BF16),
            "ctt": ctt,
        }
        for gt in range(GT):
            m[f"smp{gt}"] = smp_h[gt]
        if with_kb:
            m["kb"] = np.ascontiguousarray(
                k_b[chs][None, :] * np.float32(KSCALE)).astype(NP_BF16)
        if with_vb:
            m["vb"] = np.ascontiguousarray(
                v_b[chs][None, :] * np.float32(VSCALE)).astype(NP_BF16)
        if with_kb or with_vb:
            m["ones"] = np.ones((1, P), dtype=NP_BF16)
        in_maps.append(m)

    res = run_bass_kernel_spmd(nc, in_maps, core_ids=list(range(NCORES)),
                               trace=TRACE)
    LAST_RESULT = res

    idx = pl["idx"]
    out = np.empty((B, N, D), dtype=np.float32)
    for b in range(B):
        # unshard: sum the two head-group partials of A, then replicate
        # bin rows out to tokens (pure gather) and add the output bias.
        A = np.zeros((G, D), dtype=np.float32)
        for part in (res.results[2 * b]["aout"], res.results[2 * b + 1]["aout"]):
            for r in pl["ranges"]:
                ri, glo, ghi = r["ri"], r["glo"], r["ghi"]
                A[glo:ghi] += part[ri * P:ri * P + (ghi - glo)]
        out[b] = A[idx]
        out[b] += out_b[None, :]
    return out
